# revision 16
# baseline (speedup 1.0000x reference)
"""Trainium2 Bass kernel for nn_Graph_Enhance_model (GNN message passing).

Self-contained: hardcodes shapes B=4,F=32,H=8,O=16,D=2048, 8 cores.
Data-parallel over the 128 (b,f) frames: 16 frames per core.

v4: algebraic step-1 restructure, fp8e4m3 DoubleRow waves, fp8 human-GRU
weights, e3m4 S-GRU weights, pre-tiled weight DRAM layouts (contiguous
DMA bursts), dual HWDGE queues (sync + scalar), whh-gates computed during
phase A, PH pass interleaved with phase B, col-group-packed small-M
matmuls, half-resident gsi across both S-GRU steps.
"""

import os
import sys

for _p in ("/opt/trn_rl_repo", "/opt/pypackages"):
    if _p not in sys.path and os.path.isdir(_p):
        sys.path.append(_p)

import numpy as np
import ml_dtypes

import concourse.bass as bass
import concourse.bacc as bacc
import concourse.tile as tile
import concourse.mybir as mybir
from concourse import bass_utils
from concourse.masks import make_identity

BF16 = mybir.dt.bfloat16
F32 = mybir.dt.float32
F8 = mybir.dt.float8e4
F8E3 = mybir.dt.float8e3
AF = mybir.ActivationFunctionType
ALU = mybir.AluOpType
AX = mybir.AxisListType
DR = mybir.MatmulPerfMode.DoubleRow

NB = ml_dtypes.bfloat16
NE4 = ml_dtypes.float8_e4m3
NE3 = ml_dtypes.float8_e3m4

B, F, H, O, D = 4, 32, 8, 16, 2048
NFRAMES = B * F          # 128
NCORES = 8
FPC = NFRAMES // NCORES  # 16 frames per core
ROWS = H * O             # 128 rows per frame
KC = D // 128            # 16 K-chunks
NQ = FPC // 4            # 4 quads of 4 frames

WS = 8.0                 # fp8e4 weight scale
WS3 = 64.0               # e3m4 weight scale

_CACHE = {}


def _bc4(t, kc, q):
    """Broadcast-over-h AP: [128, 4f, 8h(stride0), 16o] of t[:, kc, q*64:(q+1)*64]."""
    base = t[:, kc, q * 64:(q + 1) * 64]
    return bass.AP(tensor=base.tensor, offset=base.offset,
                   ap=[list(base.ap[0]), [16, 4], [0, 8], [1, 16]])


def _r4(t):
    """[128, 512] -> [128, 4f, 8h, 16o]."""
    return t.rearrange("p (f h o) -> p f h o", f=4, h=8)


def _build_nc():
    nc = bacc.Bacc("TRN2", target_bir_lowering=False, debug=False, num_devices=NCORES)

    def din(name, shape, dt):
        return nc.dram_tensor(name, shape, dt, kind="ExternalInput")

    e0t = din("e0t", [NQ, 128, KC, 512], F8)
    ot = din("ot", [128, KC, FPC * O], F8)
    wnt = din("wnt", [128, KC, D // 2], F8)
    wcatA = din("wcatA", [128, KC, D // 2], F8)   # We^T (x8)
    wcatB = din("wcatB", [128, KC, D // 2], F8)   # Wl1^T (x8)
    wl1l = din("wl1l", [128, 8, D // 2], F8)
    wl1r = din("wl1r", [128, 8, D // 2], F8)
    wl2 = din("wl2", [128, 8], BF16)
    bl1td = din("bl1t", [128, 8], BF16)
    bettd = din("bett", [128, 8], BF16)
    bnttd = din("bntt", [128, 8], BF16)
    hindd = din("hind", [128, 512], BF16)
    ht8d = din("ht8", [128, KC, FPC * H], F8)
    h_rmd = din("h_rm", [FPC * H, D], F32)
    pmatd = din("pmat", [FPC * H, FPC], BF16)
    ghid = din("ghi", [12, 128, KC, 512], F8)
    ghhd = din("ghh", [12, 128, KC, 512], F8)
    ghibd = din("ghib", [1, 3 * D], BF16)
    ghhbd = din("ghhb", [1, 3 * D], BF16)
    gsid = din("gsi", [12, 128, KC, 512], F8E3)
    gshd = din("gsh", [12, 128, KC, 512], F8E3)
    gsibd = din("gsib", [1, 3 * D], BF16)
    gshbd = din("gshb", [1, 3 * D], BF16)
    scsfd = din("scsf", [128, KC, 2 * FPC], BF16)
    sc4rmd = din("sc4rm", [FPC, D], F32)
    sfrmd = din("sfrm", [FPC, D], F32)
    outp = nc.dram_tensor("outp", [FPC, D], F32, kind="ExternalOutput")

    from contextlib import ExitStack

    with tile.TileContext(nc) as tc, ExitStack() as ctx:
        glob = ctx.enter_context(tc.tile_pool(name="glob", bufs=1))
        pbias = ctx.enter_context(tc.tile_pool(name="pbias", bufs=3, side="right"))

        oi_t = glob.tile([16, 544], BF16)
        ident16 = oi_t[0:16, 0:16]
        make_identity(nc, ident16)
        ones_b = oi_t[0:1, 32:544]
        nc.vector.memset(ones_b, 1.0)
        wb_t = glob.tile([128, 544], BF16)
        wl2_sb = wb_t[:, 0:8]
        nc.sync.dma_start(out=wl2_sb, in_=wl2.ap())
        bl1t_sb = wb_t[:, 8:16]
        nc.sync.dma_start(out=bl1t_sb, in_=bl1td.ap())
        bett_sb = wb_t[:, 16:24]
        nc.sync.dma_start(out=bett_sb, in_=bettd.ap())
        bntt_sb = wb_t[:, 24:32]
        nc.sync.dma_start(out=bntt_sb, in_=bnttd.ap())
        hind_sb = wb_t[:, 32:544]
        nc.sync.dma_start(out=hind_sb, in_=hindd.ap())

        f8pair = glob.tile([128, KC, 2 * FPC * H], F8)
        msum_f8 = f8pair[:, :, 0:FPC * H]
        ht8_sb = f8pair[:, :, FPC * H:2 * FPC * H]
        nc.scalar.dma_start(out=ht8_sb, in_=ht8d.ap())
        bfpack = glob.tile([128, KC, 4 * FPC], BF16)
        scsf_sb = bfpack[:, :, 0:2 * FPC]
        nc.scalar.dma_start(out=scsf_sb, in_=scsfd.ap())
        ah_sb = bfpack[:, :, 2 * FPC:3 * FPC]
        s1t_sb = bfpack[:, :, 3 * FPC:4 * FPC]

        bw_tiles = {}

        with tc.tile_pool(name="bw", bufs=3, side="right") as bwpool, \
             tc.tile_pool(name="pghp", bufs=1) as pghp:

            ghp_sb = pghp.tile([FPC * H, 12, 512], BF16)   # whh @ H + bhh, descaled

            def bw_load(j):
                wt = bwpool.tile([128, KC, 512], F8, tag="bw")
                nc.sync.dma_start(out=wt, in_=ghid.ap()[j])
                bw_tiles[j] = wt

            # ================= Phase A =================
            with (
                tc.tile_pool(name="pal", bufs=1) as pal,
                tc.tile_pool(name="pwcat", bufs=1) as pwcat,
                tc.tile_pool(name="pa", bufs=2) as pa,
                tc.tile_pool(name="pam", bufs=2) as pam,
                tc.tile_pool(name="prelu", bufs=1) as prelu,
                tc.tile_pool(name="pa1", bufs=1) as pa1,
                tc.tile_pool(name="pav", bufs=3) as pav,
                tc.tile_pool(name="pghw", bufs=2, side="right") as pghw,
            ):
                mn_f8 = pal.tile([128, 8, FPC * O], F8)       # mn^T, unscaled
                q8rm = pal.tile([128, 2, D // 2], BF16)       # 8 * (Wl1R mn), row-major
                xu_f = pal.tile([128, KC, FPC * H], F32)      # (me0u ⊕ mnu)^T
                xu_b = pal.tile([128, KC, FPC * H], BF16)
                msum_f = pal.tile([128, KC, FPC * H], F32)    # msum^T (raw sum over o)

                wcat_sb = pwcat.tile([128, KC, D], F8)
                nc.sync.dma_start(out=wcat_sb[:, :, 0:D // 2], in_=wcatA.ap())
                nc.sync.dma_start(out=wcat_sb[:, :, D // 2:D], in_=wcatB.ap())
                wl1l_sb = pwcat.tile([128, 8, D // 2], F8)

                # ---- Phase 0: mn^T = Wn O^T + bn; Q row-major ----
                with (
                    tc.tile_pool(name="p0", bufs=1) as p0,
                    tc.tile_pool(name="p0ps", bufs=4, space="PSUM") as p0ps,
                ):
                    wnt_sb = p0.tile([128, KC, D // 2], F8)
                    nc.scalar.dma_start(out=wnt_sb, in_=wnt.ap())
                    ot_sb = p0.tile([128, KC, FPC * O], F8)
                    nc.scalar.dma_start(out=ot_sb, in_=ot.ap())
                    wl1r_sb = p0.tile([128, 8, D // 2], F8)
                    for mt in range(8):
                        pm = p0ps.tile([128, FPC * O], F32, tag="pm")
                        for i in range(8):
                            nc.tensor.matmul(pm, lhsT=wnt_sb[:, 2 * i:2 * i + 2, mt * 128:(mt + 1) * 128],
                                             rhs=ot_sb[:, 2 * i:2 * i + 2, :],
                                             perf_mode=DR, start=(i == 0), stop=(i == 7))
                        nc.scalar.activation(mn_f8[:, mt, :], pm, AF.Identity,
                                             bias=bntt_sb[:, mt:mt + 1], scale=1.0 / WS)
                    # Q row-major: q8rm[fo-chunk c] = 8 * (mn @ Wl1R.T)
                    nc.scalar.dma_start(out=wl1r_sb, in_=wl1r.ap())
                    nc.sync.dma_start(out=wl1l_sb, in_=wl1l.ap())
                    for c in range(2):
                        for n in range(2):
                            pq = p0ps.tile([128, 512], F32, tag="pq")
                            for i in range(4):
                                nc.tensor.matmul(pq, lhsT=mn_f8[:, 2 * i:2 * i + 2, c * 128:(c + 1) * 128],
                                                 rhs=wl1r_sb[:, 2 * i:2 * i + 2, n * 512:(n + 1) * 512],
                                                 perf_mode=DR, start=(i == 0), stop=(i == 3))
                            nc.scalar.copy(q8rm[:, c, n * 512:(n + 1) * 512], pq)

                with tc.tile_pool(name="paps", bufs=4, space="PSUM") as paps, \
                     tc.tile_pool(name="papss", bufs=1, space="PSUM") as papss, \
                     tc.tile_pool(name="papw", bufs=2, space="PSUM") as papw:

                    def softmax_block(relu_t, wtag):
                        pl = papss.tile([1, 512], F32, tag="pl")
                        for kc2 in range(8):
                            nc.tensor.matmul(pl, lhsT=wl2_sb[:, kc2:kc2 + 1],
                                             rhs=relu_t[:, kc2, :], start=(kc2 == 0), stop=(kc2 == 7))
                        smx = pa1.tile([1, 640], F32, tag="smx")
                        sm, rs = smx[:, 544:576], smx[:, 576:608]
                        sub = smx[:, 0:512]
                        nc.scalar.activation(sub, pl, AF.Exp)
                        ex3 = sub.rearrange("o (g i) -> o g i", i=16)
                        nc.vector.reduce_sum(sm, ex3, axis=AX.X)
                        nc.vector.reciprocal(rs, sm)
                        w_sb = wbb[0:1, 3, :]
                        nc.vector.tensor_tensor(w_sb.rearrange("o (g i) -> o g i", i=16), ex3,
                                                rs.broadcast_to((1, 32, 16)), op=ALU.mult)
                        return w_sb

                    def broadcast_w(w_sb, bidx):
                        pw = papw.tile([128, 512], F32, tag="pw")
                        nc.tensor.matmul(pw, lhsT=ones_b[0:1, 0:128], rhs=w_sb,
                                         start=True, stop=True)
                        wb = wbb[:, bidx, :]
                        nc.scalar.copy(wb, pw)
                        return wb

                    for q in range(NQ):
                        xq = pa.tile([128, KC, 512], F8, tag="xq")
                        nc.sync.dma_start(out=xq, in_=e0t.ap()[q])
                        me0t = pam.tile([128, 8, 512], F8, tag="me0t")
                        relu_sb = prelu.tile([128, 8, 512], BF16, tag="relu")
                        wbb = pa1.tile([128, 4, 512], BF16, tag="wbb")

                        # step0: me0 = We E + be
                        for mt in range(8):
                            pe = paps.tile([128, 512], F32, tag="wave")
                            for i in range(8):
                                nc.tensor.matmul(pe, lhsT=wcat_sb[:, 2 * i:2 * i + 2, mt * 128:(mt + 1) * 128],
                                                 rhs=xq[:, 2 * i:2 * i + 2, :],
                                                 perf_mode=DR, start=(i == 0), stop=(i == 7))
                            nc.scalar.activation(me0t[:, mt, :], pe, AF.Identity,
                                                 bias=bett_sb[:, mt:mt + 1], scale=1.0 / WS)
                        # step0: a0 = relu(Wl1 E + bl1)
                        for mt in range(8, 16):
                            pe = paps.tile([128, 512], F32, tag="wave")
                            for i in range(8):
                                nc.tensor.matmul(pe, lhsT=wcat_sb[:, 2 * i:2 * i + 2, mt * 128:(mt + 1) * 128],
                                                 rhs=xq[:, 2 * i:2 * i + 2, :],
                                                 perf_mode=DR, start=(i == 0), stop=(i == 7))
                            nc.scalar.activation(relu_sb[:, mt - 8, :], pe, AF.Relu,
                                                 bias=bl1t_sb[:, mt - 8:mt - 7], scale=1.0 / WS)
                        w0_sb = softmax_block(relu_sb, "w0")
                        w0b = broadcast_w(w0_sb, 0)

                        # step1: a1 = relu(w0*(P+Q) + bl1), P = Wl1L me0
                        qbase = (q % 2) * 64
                        for mt in range(8):
                            pp = paps.tile([128, 512], F32, tag="wave")
                            for i in range(4):
                                nc.tensor.matmul(pp, lhsT=wl1l_sb[:, 2 * i:2 * i + 2, mt * 128:(mt + 1) * 128],
                                                 rhs=me0t[:, 2 * i:2 * i + 2, :],
                                                 perf_mode=DR, start=(i == 0), stop=False)
                            nc.tensor.matmul(pp, lhsT=q8rm[qbase:qbase + 64, q // 2, mt * 128:(mt + 1) * 128],
                                             rhs=hind_sb[qbase:qbase + 64, :],
                                             start=False, stop=True)
                            v2 = pav.tile([128, 512], BF16, tag="v")
                            nc.vector.tensor_tensor(v2, pp, w0b, op=ALU.mult)
                            nc.scalar.activation(relu_sb[:, mt, :], v2, AF.Relu,
                                                 bias=bl1t_sb[:, mt:mt + 1], scale=1.0 / WS)
                        w1_sb = softmax_block(relu_sb, "w1")
                        w1b = broadcast_w(w1_sb, 1)
                        ub = wbb[:, 2, :]
                        nc.vector.tensor_tensor(ub, w0b, w1b, op=ALU.mult)

                        # weighted reductions over o (2-kc batched)
                        qs = slice(q * 32, (q + 1) * 32)

                        def r42(t):
                            return t.rearrange("p (k f h o) -> p k f h o", k=2, f=4, h=8)

                        for kc in range(0, 8, 2):
                            tmp = pav.tile([128, 1024], BF16, tag="v")
                            nc.vector.tensor_tensor(tmp[:, 0:512], me0t[:, kc, :], ub, op=ALU.mult)
                            nc.vector.tensor_tensor(tmp[:, 512:1024], me0t[:, kc + 1, :], ub, op=ALU.mult)
                            nc.vector.reduce_sum(xu_f[:, kc:kc + 2, qs], r42(tmp), axis=AX.X)
                        for kc in range(0, 8, 2):
                            tmp = pav.tile([128, 1024], BF16, tag="v")
                            nc.vector.tensor_tensor(_r4(tmp[:, 0:512]), _bc4(mn_f8, kc, q), _r4(ub), op=ALU.mult)
                            nc.vector.tensor_tensor(_r4(tmp[:, 512:1024]), _bc4(mn_f8, kc + 1, q), _r4(ub), op=ALU.mult)
                            nc.vector.reduce_sum(xu_f[:, 8 + kc:10 + kc, qs], r42(tmp), axis=AX.X)
                            tmp2 = pav.tile([128, 1024], BF16, tag="v")
                            nc.vector.tensor_tensor(_r4(tmp2[:, 0:512]), _bc4(mn_f8, kc, q), _r4(w1b), op=ALU.mult)
                            nc.vector.tensor_tensor(_r4(tmp2[:, 512:1024]), _bc4(mn_f8, kc + 1, q), _r4(w1b), op=ALU.mult)
                            nc.vector.reduce_sum(msum_f[:, 8 + kc:10 + kc, qs], r42(tmp2), axis=AX.X)

                        # whh-gates for 3 human-GRU blocks (weights consumed now)
                        for j in range(3 * q, 3 * q + 3):
                            wt = pghw.tile([128, KC, 512], F8, tag="ghw")
                            nc.scalar.dma_start(out=wt, in_=ghhd.ap()[j])
                            pg = papw.tile([128, 512], F32, tag="pw")
                            for i in range(8):
                                nc.tensor.matmul(pg, lhsT=ht8_sb[:, 2 * i:2 * i + 2, :],
                                                 rhs=wt[:, 2 * i:2 * i + 2, :],
                                                 perf_mode=DR, start=(i == 0), stop=False)
                            bb = pbias.tile([1, 512], BF16, tag="bias")
                            nc.sync.dma_start(out=bb, in_=ghhbd.ap()[:, j * 512:(j + 1) * 512])
                            nc.tensor.matmul(pg, lhsT=ones_b[0:1, 0:FPC * H], rhs=bb,
                                             start=False, stop=True)
                            nc.scalar.activation(ghp_sb[:, j, :], pg, AF.Copy, scale=1.0 / WS)

                    for kc in range(KC):
                        nc.vector.tensor_copy(xu_b[:, kc, :], xu_f[:, kc, :])
                    # folded msum_e = We (me0u ⊕ mnu) + be
                    for mt in range(8):
                        pf = papw.tile([128, FPC * H], F32, tag="pw")
                        for kc in range(KC):
                            nc.tensor.matmul(pf, lhsT=wcat_sb[:, kc, mt * 128:(mt + 1) * 128],
                                             rhs=xu_b[:, kc, :], start=(kc == 0), stop=(kc == KC - 1))
                        nc.scalar.activation(msum_f[:, mt, :], pf, AF.Identity,
                                             bias=bett_sb[:, mt:mt + 1], scale=1.0 / WS)
                    for kc in range(KC):
                        nc.vector.tensor_copy(msum_f8[:, kc, :], msum_f[:, kc, :])
                    bw_load(0)  # prefetch first ghi block

            # ============ Phase B (with PH pass interleaved) ============
            with tc.tile_pool(name="pcg1", bufs=1) as pcg1, \
                 tc.tile_pool(name="pci", bufs=6, side="right") as pci:
                gsi_pre = {}
                gh1_sb = pcg1.tile([FPC, 12, 512], BF16)   # whh Sc4 + bhh (descaled)
                gh2_sb = pcg1.tile([FPC, 12, 512], BF16)   # whh Sf + bhh
                with (
                    tc.tile_pool(name="pcw", bufs=3, side="right") as pcw,
                    tc.tile_pool(name="pchps", bufs=1, space="PSUM") as pchps,
                    tc.tile_pool(name="pbh", bufs=1) as pbh,
                    tc.tile_pool(name="pbt", bufs=2) as pbt,
                    tc.tile_pool(name="pbps", bufs=1, space="PSUM") as pbps,
                    tc.tile_pool(name="pbps2", bufs=2, space="PSUM") as pbps2,
                ):
                    NR = FPC * H  # 128 rows
                    h_rm = pbh.tile([NR, D], F32)
                    nc.sync.dma_start(out=h_rm, in_=h_rmd.ap())
                    pmat_sb = pbh.tile([NR, FPC], BF16)
                    nc.sync.dma_start(out=pmat_sb, in_=pmatd.ap())
                    hum_b = pbh.tile([NR, D], BF16)

                    def ph_pack(jp):
                        """gh1/gh2 = whh [Sc4|Sf] + bhh for j-blocks jp*2, jp*2+1."""
                        pch1 = pchps.tile([128, 512], F32, tag="pch1")
                        pch2 = pchps.tile([128, 512], F32, tag="pch2")
                        wts = []
                        for g in range(2):
                            j = jp * 2 + g
                            wt = pcw.tile([128, KC, 512], F8E3, tag="cw")
                            nc.scalar.dma_start(out=wt, in_=gshd.ap()[j])
                            wts.append(wt)
                        for kc in range(KC):
                            for g in range(2):
                                nc.tensor.matmul(pch1[32 * g:32 * g + 16, :],
                                                 lhsT=scsf_sb[:, kc, 0:FPC], rhs=wts[g][:, kc, :],
                                                 tile_position=(0, 32 * g),
                                                 start=(kc == 0), stop=False, skip_group_check=True)
                                nc.tensor.matmul(pch2[32 * g:32 * g + 16, :],
                                                 lhsT=scsf_sb[:, kc, FPC:2 * FPC], rhs=wts[g][:, kc, :],
                                                 tile_position=(0, 32 * g),
                                                 start=(kc == 0), stop=False, skip_group_check=True)
                        for g in range(2):
                            j = jp * 2 + g
                            bsh = pbias.tile([1, 512], BF16, tag="bias")
                            nc.sync.dma_start(out=bsh, in_=gshbd.ap()[:, j * 512:(j + 1) * 512])
                            nc.tensor.matmul(pch1[32 * g:32 * g + 16, :], lhsT=ones_b[0:1, 0:16],
                                             rhs=bsh, tile_position=(0, 32 * g),
                                             start=False, stop=True, skip_group_check=True)
                            nc.tensor.matmul(pch2[32 * g:32 * g + 16, :], lhsT=ones_b[0:1, 0:16],
                                             rhs=bsh, tile_position=(0, 32 * g),
                                             start=False, stop=True, skip_group_check=True)
                        for g in range(2):
                            j = jp * 2 + g
                            nc.scalar.activation(gh1_sb[:, j, :], pch1[32 * g:32 * g + 16, :],
                                                 AF.Copy, scale=1.0 / WS3)
                            nc.scalar.activation(gh2_sb[:, j, :], pch2[32 * g:32 * g + 16, :],
                                                 AF.Copy, scale=1.0 / WS3)

                    def gi_block(j, pt):
                        """gi-half: (wih/O) msum + bih into psum (x8)."""
                        if j not in bw_tiles:
                            bw_load(j)
                        wt = bw_tiles[j]
                        for i in range(8):
                            nc.tensor.matmul(pt, lhsT=msum_f8[:, 2 * i:2 * i + 2, :],
                                             rhs=wt[:, 2 * i:2 * i + 2, :],
                                             perf_mode=DR, start=(i == 0), stop=False)
                        bb = pbias.tile([1, 512], BF16, tag="bias")
                        nc.sync.dma_start(out=bb, in_=ghibd.ap()[:, j * 512:(j + 1) * 512])
                        nc.tensor.matmul(pt, lhsT=ones_b[0:1, 0:NR], rhs=bb,
                                         start=False, stop=True)

                    for t in range(4):
                        cols = slice(t * 512, (t + 1) * 512)
                        p_r = pbps.tile([NR, 512], F32, tag="pr")
                        gi_block(t, p_r)
                        p_z = pbps.tile([NR, 512], F32, tag="pz")
                        gi_block(4 + t, p_z)
                        p_in = pbps.tile([NR, 512], F32, tag="pin")
                        gi_block(8 + t, p_in)
                        pre_r = pbt.tile([NR, 512], F32, tag="tt")
                        nc.vector.scalar_tensor_tensor(pre_r, p_r, 1.0 / WS, ghp_sb[:, t, :],
                                                       op0=ALU.mult, op1=ALU.add)
                        r_sb = pbh.tile([NR, 512], F32, tag="r")
                        nc.scalar.activation(r_sb, pre_r, AF.Sigmoid)
                        pre_z = pbt.tile([NR, 512], F32, tag="tt")
                        nc.vector.scalar_tensor_tensor(pre_z, p_z, 1.0 / WS, ghp_sb[:, 4 + t, :],
                                                       op0=ALU.mult, op1=ALU.add)
                        z_sb = pbh.tile([NR, 512], F32, tag="z")
                        nc.scalar.activation(z_sb, pre_z, AF.Sigmoid)
                        t1 = pbt.tile([NR, 512], F32, tag="tt")
                        nc.vector.tensor_tensor(t1, r_sb, ghp_sb[:, 8 + t, :], op=ALU.mult)
                        t2 = pbt.tile([NR, 512], F32, tag="tt")
                        nc.vector.scalar_tensor_tensor(t2, p_in, 1.0 / WS, t1,
                                                       op0=ALU.mult, op1=ALU.add)
                        n_sb = pbh.tile([NR, 512], F32, tag="n")
                        nc.scalar.activation(n_sb, t2, AF.Tanh)
                        t3 = pbt.tile([NR, 512], F32, tag="tt")
                        nc.vector.tensor_tensor(t3, h_rm[:, cols], n_sb, op=ALU.subtract)
                        t4 = pbt.tile([NR, 512], F32, tag="tt")
                        nc.vector.tensor_tensor(t4, z_sb, t3, op=ALU.mult)
                        nc.vector.tensor_tensor(hum_b[:, cols], n_sb, t4, op=ALU.add)
                        if t < 3:
                            ph_pack(2 * t)
                            ph_pack(2 * t + 1)
                    for j in range(4):  # prefetch first gi1 weight blocks during late B
                        wt = pci.tile([128, KC, 512], F8E3, tag="ci")
                        eng = nc.scalar if j % 2 == 0 else nc.sync
                        eng.dma_start(out=wt, in_=gsid.ap()[j])
                        gsi_pre[j] = wt
                    for c in range(KC):
                        pah = pbps2.tile([128, FPC], F32, tag="pah")
                        nc.tensor.matmul(pah, lhsT=hum_b[:, c * 128:(c + 1) * 128], rhs=pmat_sb,
                                         start=True, stop=True)
                        nc.scalar.copy(ah_sb[:, c, :], pah)

                # ============ Phase C: two S-node GRUs ============
                with (
                    tc.tile_pool(name="pc1", bufs=1) as pc1,
                    tc.tile_pool(name="pct", bufs=2) as pct,
                    tc.tile_pool(name="pcps", bufs=2, space="PSUM") as pcps,
                    tc.tile_pool(name="pctps", bufs=2, space="PSUM") as pctps,
                ):
                    sc4rm_sb = pc1.tile([FPC, D], F32)
                    nc.sync.dma_start(out=sc4rm_sb, in_=sc4rmd.ap())
                    sfrm_sb = pc1.tile([FPC, D], F32)
                    nc.sync.dma_start(out=sfrm_sb, in_=sfrmd.ap())
                    g_sb = pc1.tile([FPC, 8, 512], BF16, tag="g")     # r,z gates
                    gn_sb = pc1.tile([FPC, 4, 512], BF16, tag="gn")   # inn
                    s1_sb = pc1.tile([FPC, D], BF16)
                    out32 = pc1.tile([FPC, D], F32)

                    def gi_pass(xt, gh_src):
                        """gi = wih x + bih (x64); g = gi/64 + gh for r,z; gi/64 for n."""
                        for jp in range(3):
                            pci_ps = pcps.tile([128, 512], F32, tag="pch")
                            wts = []
                            for g in range(4):
                                j = jp * 4 + g
                                wt = gsi_pre.pop(j, None)
                                if wt is None:
                                    wt = pci.tile([128, KC, 512], F8E3, tag="ci")
                                    eng = nc.scalar if j % 2 == 0 else nc.sync
                                    eng.dma_start(out=wt, in_=gsid.ap()[j])
                                wts.append(wt)
                            for kc in range(KC):
                                for g in range(4):
                                    nc.tensor.matmul(pci_ps[32 * g:32 * g + 16, :],
                                                     lhsT=xt[:, kc, :], rhs=wts[g][:, kc, :],
                                                     tile_position=(0, 32 * g),
                                                     start=(kc == 0), stop=False, skip_group_check=True)
                            for g in range(4):
                                j = jp * 4 + g
                                bsi = pbias.tile([1, 512], BF16, tag="bias")
                                nc.sync.dma_start(out=bsi, in_=gsibd.ap()[:, j * 512:(j + 1) * 512])
                                nc.tensor.matmul(pci_ps[32 * g:32 * g + 16, :], lhsT=ones_b[0:1, 0:16],
                                                 rhs=bsi, tile_position=(0, 32 * g),
                                                 start=False, stop=True, skip_group_check=True)
                            for g in range(4):
                                j = jp * 4 + g
                                if j < 8:
                                    nc.vector.scalar_tensor_tensor(
                                        g_sb[:, j, :], pci_ps[32 * g:32 * g + 16, :], 1.0 / WS3,
                                        gh_src[:, j, :], op0=ALU.mult, op1=ALU.add)
                                else:
                                    nc.scalar.activation(gn_sb[:, j - 8, :], pci_ps[32 * g:32 * g + 16, :],
                                                         AF.Copy, scale=1.0 / WS3)

                    def s_elementwise(gh_src, hprev, outt):
                        for t in range(4):
                            cols = slice(t * 512, (t + 1) * 512)
                            r1 = pc1.tile([FPC, 512], F32, tag="c_r")
                            nc.scalar.activation(r1, g_sb[:, t, :], AF.Sigmoid)
                            z1 = pc1.tile([FPC, 512], F32, tag="c_z")
                            nc.scalar.activation(z1, g_sb[:, 4 + t, :], AF.Sigmoid)
                            u1 = pct.tile([FPC, 512], F32, tag="cu")
                            nc.vector.tensor_tensor(u1, r1, gh_src[:, 8 + t, :], op=ALU.mult)
                            u2 = pct.tile([FPC, 512], F32, tag="cu")
                            nc.vector.tensor_tensor(u2, u1, gn_sb[:, t, :], op=ALU.add)
                            n1 = pc1.tile([FPC, 512], F32, tag="c_n")
                            nc.scalar.activation(n1, u2, AF.Tanh)
                            u3 = pct.tile([FPC, 512], F32, tag="cu")
                            nc.vector.tensor_tensor(u3, hprev[:, cols], n1, op=ALU.subtract)
                            u4 = pct.tile([FPC, 512], F32, tag="cu")
                            nc.vector.tensor_tensor(u4, z1, u3, op=ALU.mult)
                            nc.vector.tensor_tensor(outt[:, cols], n1, u4, op=ALU.add)

                    gi_pass(ah_sb, gh1_sb)
                    s_elementwise(gh1_sb, sc4rm_sb, s1_sb)
                    for c in range(KC):
                        ptp = pctps.tile([128, 16], BF16, tag="tp")
                        nc.tensor.transpose(ptp, s1_sb[:, c * 128:(c + 1) * 128], ident16)
                        nc.scalar.copy(s1t_sb[:, c, :], ptp)
                    gi_pass(s1t_sb, gh2_sb)
                    s_elementwise(gh2_sb, sfrm_sb, out32)
                    nc.sync.dma_start(out=outp.ap(), in_=out32)

    nc.compile()
    return nc


def _tile_w(WT, blocks):
    """[2048, blocks*512] -> [blocks, 128, 16, 512] (pre-tiled for contiguous DMA)."""
    return np.ascontiguousarray(
        WT.reshape(16, 128, blocks, 512).transpose(2, 1, 0, 3))


def _tile_k(WT):
    """[2048, N] -> [128, 16, N]."""
    n = WT.shape[1]
    return np.ascontiguousarray(WT.reshape(16, 128, n).transpose(1, 0, 2))


def _make_hind():
    """h-broadcast indicator: hind[p, f*128+h*16+o] = (p%64 == f*16+o)."""
    m = np.zeros((128, 512), dtype=NB)
    for n in range(512):
        f, o = n // 128, n % 16
        m[f * 16 + o, n] = 1.0
        m[64 + f * 16 + o, n] = 1.0
    return m


def _prep_in_maps(inputs):
    E = np.ascontiguousarray(inputs["H_O_edges"].reshape(NFRAMES, ROWS, D))
    On = inputs["O_nodes"].reshape(NFRAMES, O, D)
    Hn = inputs["H_nodes"].reshape(NFRAMES, H, D)
    Sc4 = inputs["S_node_C4"].reshape(NFRAMES, D)
    Sf = np.ascontiguousarray(inputs["final_S_node"].transpose(0, 2, 1)).reshape(NFRAMES, D)

    We, Wl1, Wn = inputs["We"], inputs["Wl1"], inputs["Wn"]

    shared = {
        "wcatA": _tile_k((We * WS).T.astype(NE4)),
        "wcatB": _tile_k((Wl1 * WS).T.astype(NE4)),
        "wl1l": np.ascontiguousarray(
            (Wl1[:, :D // 2] * WS).T.astype(NE4).reshape(8, 128, D // 2).transpose(1, 0, 2)),
        "wl1r": np.ascontiguousarray(
            (Wl1[:, D // 2:] * WS).T.astype(NE4).reshape(8, 128, D // 2).transpose(1, 0, 2)),
        "wnt": _tile_k((Wn * WS).T.astype(NE4)),
        "wl2": np.ascontiguousarray(inputs["Wl2"][0].reshape(8, 128).T).astype(NB),
        "bl1t": np.ascontiguousarray(inputs["bl1"].reshape(8, 128).T).astype(NB),
        "bett": np.ascontiguousarray(inputs["be"].reshape(8, 128).T).astype(NB),
        "bntt": np.ascontiguousarray(inputs["bn"].reshape(8, 128).T).astype(NB),
        "hind": _make_hind(),
        "pmat": np.ascontiguousarray(np.kron(np.eye(FPC), np.ones((H, 1))) / H).astype(NB),
        "ghi": _tile_w((inputs["gh_wih"] * (WS / O)).T.astype(NE4), 12),
        "ghh": _tile_w((inputs["gh_whh"] * WS).T.astype(NE4), 12),
        "ghib": (inputs["gh_bih"] * WS)[None, :].astype(NB),
        "ghhb": (inputs["gh_bhh"] * WS)[None, :].astype(NB),
        "gsi": _tile_w((inputs["gs_wih"] * WS3).T.astype(NE3), 12),
        "gsh": _tile_w((inputs["gs_whh"] * WS3).T.astype(NE3), 12),
        "gsib": (inputs["gs_bih"] * WS3)[None, :].astype(NB),
        "gshb": (inputs["gs_bhh"] * WS3)[None, :].astype(NB),
    }

    in_maps = []
    for c in range(NCORES):
        fr = slice(c * FPC, (c + 1) * FPC)
        Ec = E[fr]  # [16, 128, 2048]
        e0t = np.ascontiguousarray(
            Ec.reshape(NQ, 4, ROWS, D).transpose(0, 3, 1, 2)
            .reshape(NQ, 16, 128, 512).transpose(0, 2, 1, 3)).astype(NE4)
        m = dict(shared)
        m.update({
            "e0t": e0t,
            "ot": _tile_k(On[fr].reshape(FPC * O, D).T.astype(NE4)),
            "ht8": _tile_k(Hn[fr].reshape(FPC * H, D).T.astype(NE4)),
            "h_rm": np.ascontiguousarray(Hn[fr].reshape(FPC * H, D)).astype(np.float32),
            "scsf": _tile_k(np.concatenate([Sc4[fr].T, Sf[fr].T], axis=1).astype(NB)),
            "sc4rm": np.ascontiguousarray(Sc4[fr]).astype(np.float32),
            "sfrm": np.ascontiguousarray(Sf[fr]).astype(np.float32),
        })
        in_maps.append(m)
    return in_maps


LAST_RESULT = None


def kernel(**inputs):
    global LAST_RESULT
    if "nc" not in _CACHE:
        _CACHE["nc"] = _build_nc()
    nc = _CACHE["nc"]
    in_maps = _prep_in_maps(inputs)
    trace = os.environ.get("KERNEL_TRACE", "0") == "1"
    res = bass_utils.run_bass_kernel_spmd(
        nc, in_maps, core_ids=list(range(NCORES)), trace=trace)
    LAST_RESULT = res
    out = np.concatenate([res.results[c]["outp"] for c in range(NCORES)], axis=0)
    return np.ascontiguousarray(out.reshape(B, F, D)).astype(np.float32)


# revision 17
# speedup vs baseline: 1.0673x; 1.0673x over previous
"""Trainium2 Bass kernel for nn_Graph_Enhance_model (GNN message passing).

Self-contained: hardcodes shapes B=4,F=32,H=8,O=16,D=2048, 8 cores.
Data-parallel over the 128 (b,f) frames: 16 frames per core.

v4: algebraic step-1 restructure, fp8e4m3 DoubleRow waves, fp8 human-GRU
weights, e3m4 S-GRU weights, pre-tiled weight DRAM layouts (contiguous
DMA bursts), dual HWDGE queues (sync + scalar), whh-gates computed during
phase A, PH pass interleaved with phase B, col-group-packed small-M
matmuls, half-resident gsi across both S-GRU steps.
"""

import os
import sys

for _p in ("/opt/trn_rl_repo", "/opt/pypackages"):
    if _p not in sys.path and os.path.isdir(_p):
        sys.path.append(_p)

import numpy as np
import ml_dtypes

import concourse.bass as bass
import concourse.bacc as bacc
import concourse.tile as tile
import concourse.mybir as mybir
from concourse import bass_utils
from concourse.masks import make_identity

BF16 = mybir.dt.bfloat16
F32 = mybir.dt.float32
F8 = mybir.dt.float8e4
F8E3 = mybir.dt.float8e3
AF = mybir.ActivationFunctionType
ALU = mybir.AluOpType
AX = mybir.AxisListType
DR = mybir.MatmulPerfMode.DoubleRow

NB = ml_dtypes.bfloat16
NE4 = ml_dtypes.float8_e4m3
NE3 = ml_dtypes.float8_e3m4

B, F, H, O, D = 4, 32, 8, 16, 2048
NFRAMES = B * F          # 128
NCORES = 8
FPC = NFRAMES // NCORES  # 16 frames per core
ROWS = H * O             # 128 rows per frame
KC = D // 128            # 16 K-chunks
NQ = FPC // 4            # 4 quads of 4 frames

WS = 8.0                 # fp8e4 weight scale
WS3 = 64.0               # e3m4 weight scale

_CACHE = {}


def _bc4(t, kc, q):
    """Broadcast-over-h AP: [128, 4f, 8h(stride0), 16o] of t[:, kc, q*64:(q+1)*64]."""
    base = t[:, kc, q * 64:(q + 1) * 64]
    return bass.AP(tensor=base.tensor, offset=base.offset,
                   ap=[list(base.ap[0]), [16, 4], [0, 8], [1, 16]])


def _r4(t):
    """[128, 512] -> [128, 4f, 8h, 16o]."""
    return t.rearrange("p (f h o) -> p f h o", f=4, h=8)


def _build_nc():
    nc = bacc.Bacc("TRN2", target_bir_lowering=False, debug=False, num_devices=NCORES)

    def din(name, shape, dt):
        return nc.dram_tensor(name, shape, dt, kind="ExternalInput")

    e0t = din("e0t", [NQ, 128, KC, 512], F8)
    ot = din("ot", [128, KC, FPC * O], F8)
    wnt = din("wnt", [128, KC, D // 2], F8)
    wcatA = din("wcatA", [128, KC, D // 2], F8)   # We^T (x8)
    wcatB = din("wcatB", [128, KC, D // 2], F8)   # Wl1^T (x8)
    wl1l = din("wl1l", [128, 8, D // 2], F8)
    wl1r = din("wl1r", [128, 8, D // 2], F8)
    wl2 = din("wl2", [128, 8], BF16)
    bl1td = din("bl1t", [128, 8], BF16)
    bettd = din("bett", [128, 8], BF16)
    bnttd = din("bntt", [128, 8], BF16)
    hindd = din("hind", [128, 512], BF16)
    ht8d = din("ht8", [128, KC, FPC * H], F8)
    h_rmd = din("h_rm", [FPC * H, D], F32)
    pmatd = din("pmat", [FPC * H, FPC], BF16)
    ghid = din("ghi", [12, 128, KC, 512], F8)
    ghhd = din("ghh", [12, 128, KC, 512], F8)
    ghibd = din("ghib", [1, 3 * D], BF16)
    ghhbd = din("ghhb", [1, 3 * D], BF16)
    gsid = din("gsi", [12, 128, KC, 512], F8E3)
    gshd = din("gsh", [12, 128, KC, 512], F8E3)
    gsibd = din("gsib", [1, 3 * D], BF16)
    gshbd = din("gshb", [1, 3 * D], BF16)
    scsfd = din("scsf", [128, KC, 2 * FPC], BF16)
    sc4rmd = din("sc4rm", [FPC, D], F32)
    sfrmd = din("sfrm", [FPC, D], F32)
    outp = nc.dram_tensor("outp", [FPC, D], F32, kind="ExternalOutput")

    from contextlib import ExitStack

    with tile.TileContext(nc) as tc, ExitStack() as ctx:
        glob = ctx.enter_context(tc.tile_pool(name="glob", bufs=1))
        pbias = ctx.enter_context(tc.tile_pool(name="pbias", bufs=3, side="right"))

        oi_t = glob.tile([16, 544], BF16)
        ident16 = oi_t[0:16, 0:16]
        make_identity(nc, ident16)
        ones_b = oi_t[0:1, 32:544]
        nc.vector.memset(ones_b, 1.0)
        wb_t = glob.tile([128, 544], BF16)
        wl2_sb = wb_t[:, 0:8]
        nc.sync.dma_start(out=wl2_sb, in_=wl2.ap())
        bl1t_sb = wb_t[:, 8:16]
        nc.sync.dma_start(out=bl1t_sb, in_=bl1td.ap())
        bett_sb = wb_t[:, 16:24]
        nc.sync.dma_start(out=bett_sb, in_=bettd.ap())
        bntt_sb = wb_t[:, 24:32]
        nc.sync.dma_start(out=bntt_sb, in_=bnttd.ap())
        hind_sb = wb_t[:, 32:544]
        nc.sync.dma_start(out=hind_sb, in_=hindd.ap())

        f8pair = glob.tile([128, KC, 2 * FPC * H], F8)
        msum_f8 = f8pair[:, :, 0:FPC * H]
        ht8_sb = f8pair[:, :, FPC * H:2 * FPC * H]
        nc.scalar.dma_start(out=ht8_sb, in_=ht8d.ap())
        bfpack = glob.tile([128, KC, 4 * FPC], BF16)
        scsf_sb = bfpack[:, :, 0:2 * FPC]
        nc.scalar.dma_start(out=scsf_sb, in_=scsfd.ap())
        ah_sb = bfpack[:, :, 2 * FPC:3 * FPC]
        s1t_sb = bfpack[:, :, 3 * FPC:4 * FPC]

        bw_tiles = {}

        with tc.tile_pool(name="bw", bufs=3, side="right") as bwpool, \
             tc.tile_pool(name="pghp", bufs=1) as pghp:

            ghp_sb = pghp.tile([FPC * H, 12, 512], BF16)   # whh @ H + bhh, descaled

            def bw_load(j):
                wt = bwpool.tile([128, KC, 512], F8, tag="bw")
                nc.sync.dma_start(out=wt, in_=ghid.ap()[j])
                bw_tiles[j] = wt

            # ================= Phase A =================
            with (
                tc.tile_pool(name="pal", bufs=1) as pal,
                tc.tile_pool(name="pwcat", bufs=1) as pwcat,
                tc.tile_pool(name="pa", bufs=2) as pa,
                tc.tile_pool(name="pam", bufs=2) as pam,
                tc.tile_pool(name="prelu", bufs=1) as prelu,
                tc.tile_pool(name="pa1", bufs=1) as pa1,
                tc.tile_pool(name="pav", bufs=3) as pav,
                tc.tile_pool(name="pghw", bufs=2, side="right") as pghw,
            ):
                mn_f8 = pal.tile([128, 8, FPC * O], F8)       # mn^T, unscaled
                q8rm = pal.tile([128, 2, D // 2], BF16)       # 8 * (Wl1R mn), row-major
                xu_f = pal.tile([128, KC, FPC * H], F32)      # (me0u ⊕ mnu)^T
                xu_b = pal.tile([128, KC, FPC * H], BF16)
                msum_f = pal.tile([128, KC, FPC * H], F32)    # msum^T (raw sum over o)

                wcat_sb = pwcat.tile([128, KC, D], F8)
                nc.sync.dma_start(out=wcat_sb[:, :, 0:D // 2], in_=wcatA.ap())
                nc.sync.dma_start(out=wcat_sb[:, :, D // 2:D], in_=wcatB.ap())
                wl1l_sb = pwcat.tile([128, 8, D // 2], F8)

                # ---- Phase 0: mn^T = Wn O^T + bn; Q row-major ----
                with (
                    tc.tile_pool(name="p0", bufs=1) as p0,
                    tc.tile_pool(name="p0ps", bufs=4, space="PSUM") as p0ps,
                ):
                    wnt_sb = p0.tile([128, KC, D // 2], F8)
                    nc.scalar.dma_start(out=wnt_sb, in_=wnt.ap())
                    ot_sb = p0.tile([128, KC, FPC * O], F8)
                    nc.scalar.dma_start(out=ot_sb, in_=ot.ap())
                    wl1r_sb = p0.tile([128, 8, D // 2], F8)
                    for mt in range(8):
                        pm = p0ps.tile([128, FPC * O], F32, tag="pm")
                        for i in range(8):
                            nc.tensor.matmul(pm, lhsT=wnt_sb[:, 2 * i:2 * i + 2, mt * 128:(mt + 1) * 128],
                                             rhs=ot_sb[:, 2 * i:2 * i + 2, :],
                                             perf_mode=DR, start=(i == 0), stop=(i == 7))
                        nc.scalar.activation(mn_f8[:, mt, :], pm, AF.Identity,
                                             bias=bntt_sb[:, mt:mt + 1], scale=1.0 / WS)
                    # Q row-major: q8rm[fo-chunk c] = 8 * (mn @ Wl1R.T)
                    nc.scalar.dma_start(out=wl1r_sb, in_=wl1r.ap())
                    nc.sync.dma_start(out=wl1l_sb, in_=wl1l.ap())
                    for c in range(2):
                        for n in range(2):
                            pq = p0ps.tile([128, 512], F32, tag="pq")
                            for i in range(4):
                                nc.tensor.matmul(pq, lhsT=mn_f8[:, 2 * i:2 * i + 2, c * 128:(c + 1) * 128],
                                                 rhs=wl1r_sb[:, 2 * i:2 * i + 2, n * 512:(n + 1) * 512],
                                                 perf_mode=DR, start=(i == 0), stop=(i == 3))
                            nc.scalar.copy(q8rm[:, c, n * 512:(n + 1) * 512], pq)

                with tc.tile_pool(name="paps", bufs=4, space="PSUM") as paps, \
                     tc.tile_pool(name="papss", bufs=1, space="PSUM") as papss, \
                     tc.tile_pool(name="papw", bufs=2, space="PSUM") as papw:

                    def softmax_block(relu_t, wtag):
                        pl = papss.tile([1, 512], F32, tag="pl")
                        for kc2 in range(8):
                            nc.tensor.matmul(pl, lhsT=wl2_sb[:, kc2:kc2 + 1],
                                             rhs=relu_t[:, kc2, :], start=(kc2 == 0), stop=(kc2 == 7))
                        smx = pa1.tile([1, 640], F32, tag="smx")
                        sm, rs = smx[:, 544:576], smx[:, 576:608]
                        sub = smx[:, 0:512]
                        nc.scalar.activation(sub, pl, AF.Exp)
                        ex3 = sub.rearrange("o (g i) -> o g i", i=16)
                        nc.vector.reduce_sum(sm, ex3, axis=AX.X)
                        nc.vector.reciprocal(rs, sm)
                        w_sb = wbb[0:1, 3, :]
                        nc.vector.tensor_tensor(w_sb.rearrange("o (g i) -> o g i", i=16), ex3,
                                                rs.broadcast_to((1, 32, 16)), op=ALU.mult)
                        return w_sb

                    def broadcast_w(w_sb, bidx):
                        pw = papw.tile([128, 512], F32, tag="pw")
                        nc.tensor.matmul(pw, lhsT=ones_b[0:1, 0:128], rhs=w_sb,
                                         start=True, stop=True)
                        wb = wbb[:, bidx, :]
                        nc.scalar.copy(wb, pw)
                        return wb

                    for q in range(NQ):
                        xq = pa.tile([128, KC, 512], F8, tag="xq")
                        nc.sync.dma_start(out=xq, in_=e0t.ap()[q])
                        me0t = pam.tile([128, 8, 512], F8, tag="me0t")
                        relu_sb = prelu.tile([128, 8, 512], BF16, tag="relu")
                        wbb = pa1.tile([128, 4, 512], BF16, tag="wbb")

                        # step0: me0 = We E + be
                        for mt in range(8):
                            pe = paps.tile([128, 512], F32, tag="wave")
                            for i in range(8):
                                nc.tensor.matmul(pe, lhsT=wcat_sb[:, 2 * i:2 * i + 2, mt * 128:(mt + 1) * 128],
                                                 rhs=xq[:, 2 * i:2 * i + 2, :],
                                                 perf_mode=DR, start=(i == 0), stop=(i == 7))
                            nc.scalar.activation(me0t[:, mt, :], pe, AF.Identity,
                                                 bias=bett_sb[:, mt:mt + 1], scale=1.0 / WS)
                        # step0: a0 = relu(Wl1 E + bl1)
                        for mt in range(8, 16):
                            pe = paps.tile([128, 512], F32, tag="wave")
                            for i in range(8):
                                nc.tensor.matmul(pe, lhsT=wcat_sb[:, 2 * i:2 * i + 2, mt * 128:(mt + 1) * 128],
                                                 rhs=xq[:, 2 * i:2 * i + 2, :],
                                                 perf_mode=DR, start=(i == 0), stop=(i == 7))
                            nc.scalar.activation(relu_sb[:, mt - 8, :], pe, AF.Relu,
                                                 bias=bl1t_sb[:, mt - 8:mt - 7], scale=1.0 / WS)
                        w0_sb = softmax_block(relu_sb, "w0")
                        w0b = broadcast_w(w0_sb, 0)

                        # step1: a1 = relu(w0*(P+Q) + bl1), P = Wl1L me0
                        qbase = (q % 2) * 64
                        for mt in range(8):
                            pp = paps.tile([128, 512], F32, tag="wave")
                            for i in range(4):
                                nc.tensor.matmul(pp, lhsT=wl1l_sb[:, 2 * i:2 * i + 2, mt * 128:(mt + 1) * 128],
                                                 rhs=me0t[:, 2 * i:2 * i + 2, :],
                                                 perf_mode=DR, start=(i == 0), stop=False)
                            nc.tensor.matmul(pp, lhsT=q8rm[qbase:qbase + 64, q // 2, mt * 128:(mt + 1) * 128],
                                             rhs=hind_sb[qbase:qbase + 64, :],
                                             start=False, stop=True)
                            v2 = pav.tile([128, 512], BF16, tag="v")
                            nc.vector.tensor_tensor(v2, pp, w0b, op=ALU.mult)
                            nc.scalar.activation(relu_sb[:, mt, :], v2, AF.Relu,
                                                 bias=bl1t_sb[:, mt:mt + 1], scale=1.0 / WS)
                        w1_sb = softmax_block(relu_sb, "w1")
                        w1b = broadcast_w(w1_sb, 1)
                        ub = wbb[:, 2, :]
                        nc.vector.tensor_tensor(ub, w0b, w1b, op=ALU.mult)

                        # weighted reductions over o (2-kc batched)
                        qs = slice(q * 32, (q + 1) * 32)

                        def r42(t):
                            return t.rearrange("p (k f h o) -> p k f h o", k=2, f=4, h=8)

                        for kc in range(0, 8, 2):
                            tmp = pav.tile([128, 1024], BF16, tag="v")
                            nc.vector.tensor_tensor(tmp[:, 0:512], me0t[:, kc, :], ub, op=ALU.mult)
                            nc.vector.tensor_tensor(tmp[:, 512:1024], me0t[:, kc + 1, :], ub, op=ALU.mult)
                            nc.vector.reduce_sum(xu_f[:, kc:kc + 2, qs], r42(tmp), axis=AX.X)
                        for kc in range(0, 8, 2):
                            tmp = pav.tile([128, 1024], BF16, tag="v")
                            nc.vector.tensor_tensor(_r4(tmp[:, 0:512]), _bc4(mn_f8, kc, q), _r4(ub), op=ALU.mult)
                            nc.vector.tensor_tensor(_r4(tmp[:, 512:1024]), _bc4(mn_f8, kc + 1, q), _r4(ub), op=ALU.mult)
                            nc.vector.reduce_sum(xu_f[:, 8 + kc:10 + kc, qs], r42(tmp), axis=AX.X)
                            tmp2 = pav.tile([128, 1024], BF16, tag="v")
                            nc.vector.tensor_tensor(_r4(tmp2[:, 0:512]), _bc4(mn_f8, kc, q), _r4(w1b), op=ALU.mult)
                            nc.vector.tensor_tensor(_r4(tmp2[:, 512:1024]), _bc4(mn_f8, kc + 1, q), _r4(w1b), op=ALU.mult)
                            nc.vector.reduce_sum(msum_f[:, 8 + kc:10 + kc, qs], r42(tmp2), axis=AX.X)

                        # whh-gates for 3 human-GRU blocks (weights consumed now)
                        for j in range(3 * q, 3 * q + 3):
                            wt = pghw.tile([128, KC, 512], F8, tag="ghw")
                            nc.scalar.dma_start(out=wt, in_=ghhd.ap()[j])
                            pg = papw.tile([128, 512], F32, tag="pw")
                            for i in range(8):
                                nc.tensor.matmul(pg, lhsT=ht8_sb[:, 2 * i:2 * i + 2, :],
                                                 rhs=wt[:, 2 * i:2 * i + 2, :],
                                                 perf_mode=DR, start=(i == 0), stop=False)
                            bb = pbias.tile([1, 512], BF16, tag="bias")
                            nc.sync.dma_start(out=bb, in_=ghhbd.ap()[:, j * 512:(j + 1) * 512])
                            nc.tensor.matmul(pg, lhsT=ones_b[0:1, 0:FPC * H], rhs=bb,
                                             start=False, stop=True)
                            nc.scalar.activation(ghp_sb[:, j, :], pg, AF.Copy, scale=1.0 / WS)

                    for kc in range(KC):
                        nc.vector.tensor_copy(xu_b[:, kc, :], xu_f[:, kc, :])
                    # folded msum_e = We (me0u ⊕ mnu) + be
                    for mt in range(8):
                        pf = papw.tile([128, FPC * H], F32, tag="pw")
                        for kc in range(KC):
                            nc.tensor.matmul(pf, lhsT=wcat_sb[:, kc, mt * 128:(mt + 1) * 128],
                                             rhs=xu_b[:, kc, :], start=(kc == 0), stop=(kc == KC - 1))
                        nc.scalar.activation(msum_f[:, mt, :], pf, AF.Identity,
                                             bias=bett_sb[:, mt:mt + 1], scale=1.0 / WS)
                    for kc in range(KC):
                        nc.vector.tensor_copy(msum_f8[:, kc, :], msum_f[:, kc, :])
                    bw_load(0)  # prefetch first ghi block

            # ============ Phase B (with PH pass interleaved) ============
            with tc.tile_pool(name="pcg1", bufs=1) as pcg1, \
                 tc.tile_pool(name="pci", bufs=6, side="right") as pci:
                gsi_pre = {}
                gh1_sb = pcg1.tile([FPC, 12, 512], BF16)   # whh Sc4 + bhh (descaled)
                gh2_sb = pcg1.tile([FPC, 12, 512], BF16)   # whh Sf + bhh
                with (
                    tc.tile_pool(name="pcw", bufs=3, side="right") as pcw,
                    tc.tile_pool(name="pchps", bufs=1, space="PSUM") as pchps,
                    tc.tile_pool(name="pbh", bufs=1) as pbh,
                    tc.tile_pool(name="pbt", bufs=2) as pbt,
                    tc.tile_pool(name="pbps", bufs=1, space="PSUM") as pbps,
                    tc.tile_pool(name="pbps2", bufs=2, space="PSUM") as pbps2,
                ):
                    NR = FPC * H  # 128 rows
                    h_rm = pbh.tile([NR, D], F32)
                    nc.sync.dma_start(out=h_rm, in_=h_rmd.ap())
                    pmat_sb = pbh.tile([NR, FPC], BF16)
                    nc.sync.dma_start(out=pmat_sb, in_=pmatd.ap())
                    hum_b = pbh.tile([NR, D], BF16)

                    def ph_pack(jp):
                        """gh1/gh2 = whh [Sc4|Sf] + bhh for j-blocks jp*2, jp*2+1."""
                        pch1 = pchps.tile([128, 512], F32, tag="pch1")
                        pch2 = pchps.tile([128, 512], F32, tag="pch2")
                        wts = []
                        for g in range(2):
                            j = jp * 2 + g
                            wt = pcw.tile([128, KC, 512], F8E3, tag="cw")
                            nc.scalar.dma_start(out=wt, in_=gshd.ap()[j])
                            wts.append(wt)
                        for kc in range(KC):
                            for g in range(2):
                                nc.tensor.matmul(pch1[32 * g:32 * g + 16, :],
                                                 lhsT=scsf_sb[:, kc, 0:FPC], rhs=wts[g][:, kc, :],
                                                 tile_position=(0, 32 * g),
                                                 start=(kc == 0), stop=False, skip_group_check=True)
                                nc.tensor.matmul(pch2[32 * g:32 * g + 16, :],
                                                 lhsT=scsf_sb[:, kc, FPC:2 * FPC], rhs=wts[g][:, kc, :],
                                                 tile_position=(0, 32 * g),
                                                 start=(kc == 0), stop=False, skip_group_check=True)
                        for g in range(2):
                            j = jp * 2 + g
                            bsh = pbias.tile([1, 512], BF16, tag="bias")
                            nc.sync.dma_start(out=bsh, in_=gshbd.ap()[:, j * 512:(j + 1) * 512])
                            nc.tensor.matmul(pch1[32 * g:32 * g + 16, :], lhsT=ones_b[0:1, 0:16],
                                             rhs=bsh, tile_position=(0, 32 * g),
                                             start=False, stop=True, skip_group_check=True)
                            nc.tensor.matmul(pch2[32 * g:32 * g + 16, :], lhsT=ones_b[0:1, 0:16],
                                             rhs=bsh, tile_position=(0, 32 * g),
                                             start=False, stop=True, skip_group_check=True)
                        for g in range(2):
                            j = jp * 2 + g
                            nc.scalar.activation(gh1_sb[:, j, :], pch1[32 * g:32 * g + 16, :],
                                                 AF.Copy, scale=1.0 / WS3)
                            nc.scalar.activation(gh2_sb[:, j, :], pch2[32 * g:32 * g + 16, :],
                                                 AF.Copy, scale=1.0 / WS3)

                    def gi_block(j, pt):
                        """gi-half: (wih/O) msum + bih into psum (x8)."""
                        if j not in bw_tiles:
                            bw_load(j)
                        wt = bw_tiles[j]
                        for i in range(8):
                            nc.tensor.matmul(pt, lhsT=msum_f8[:, 2 * i:2 * i + 2, :],
                                             rhs=wt[:, 2 * i:2 * i + 2, :],
                                             perf_mode=DR, start=(i == 0), stop=False)
                        bb = pbias.tile([1, 512], BF16, tag="bias")
                        nc.sync.dma_start(out=bb, in_=ghibd.ap()[:, j * 512:(j + 1) * 512])
                        nc.tensor.matmul(pt, lhsT=ones_b[0:1, 0:NR], rhs=bb,
                                         start=False, stop=True)

                    for t in range(4):
                        cols = slice(t * 512, (t + 1) * 512)
                        p_r = pbps.tile([NR, 512], F32, tag="pr")
                        gi_block(t, p_r)
                        p_z = pbps.tile([NR, 512], F32, tag="pz")
                        gi_block(4 + t, p_z)
                        p_in = pbps.tile([NR, 512], F32, tag="pin")
                        gi_block(8 + t, p_in)
                        pre_r = pbt.tile([NR, 512], F32, tag="tt")
                        nc.vector.scalar_tensor_tensor(pre_r, p_r, 1.0 / WS, ghp_sb[:, t, :],
                                                       op0=ALU.mult, op1=ALU.add)
                        r_sb = pbh.tile([NR, 512], F32, tag="r")
                        nc.scalar.activation(r_sb, pre_r, AF.Sigmoid)
                        pre_z = pbt.tile([NR, 512], F32, tag="tt")
                        nc.vector.scalar_tensor_tensor(pre_z, p_z, 1.0 / WS, ghp_sb[:, 4 + t, :],
                                                       op0=ALU.mult, op1=ALU.add)
                        z_sb = pbh.tile([NR, 512], F32, tag="z")
                        nc.scalar.activation(z_sb, pre_z, AF.Sigmoid)
                        t1 = pbt.tile([NR, 512], F32, tag="tt")
                        nc.vector.tensor_tensor(t1, r_sb, ghp_sb[:, 8 + t, :], op=ALU.mult)
                        t2 = pbt.tile([NR, 512], F32, tag="tt")
                        nc.vector.scalar_tensor_tensor(t2, p_in, 1.0 / WS, t1,
                                                       op0=ALU.mult, op1=ALU.add)
                        n_sb = pbh.tile([NR, 512], F32, tag="n")
                        nc.scalar.activation(n_sb, t2, AF.Tanh)
                        t3 = pbt.tile([NR, 512], F32, tag="tt")
                        nc.vector.tensor_tensor(t3, h_rm[:, cols], n_sb, op=ALU.subtract)
                        t4 = pbt.tile([NR, 512], F32, tag="tt")
                        nc.vector.tensor_tensor(t4, z_sb, t3, op=ALU.mult)
                        nc.vector.tensor_tensor(hum_b[:, cols], n_sb, t4, op=ALU.add)
                        if t < 3:
                            ph_pack(2 * t)
                            ph_pack(2 * t + 1)
                    for j in range(4):  # prefetch first gi1 weight blocks during late B
                        wt = pci.tile([128, KC, 512], F8E3, tag="ci")
                        nc.scalar.dma_start(out=wt, in_=gsid.ap()[j])
                        gsi_pre[j] = wt
                    for c in range(KC):
                        pah = pbps2.tile([128, FPC], F32, tag="pah")
                        nc.tensor.matmul(pah, lhsT=hum_b[:, c * 128:(c + 1) * 128], rhs=pmat_sb,
                                         start=True, stop=True)
                        nc.scalar.copy(ah_sb[:, c, :], pah)

                # ============ Phase C: two S-node GRUs ============
                with (
                    tc.tile_pool(name="pc1", bufs=1) as pc1,
                    tc.tile_pool(name="pct", bufs=2) as pct,
                    tc.tile_pool(name="pcps", bufs=2, space="PSUM") as pcps,
                    tc.tile_pool(name="pctps", bufs=2, space="PSUM") as pctps,
                ):
                    sc4rm_sb = pc1.tile([FPC, D], F32)
                    nc.sync.dma_start(out=sc4rm_sb, in_=sc4rmd.ap())
                    sfrm_sb = pc1.tile([FPC, D], F32)
                    nc.sync.dma_start(out=sfrm_sb, in_=sfrmd.ap())
                    g_sb = pc1.tile([FPC, 8, 512], BF16, tag="g")     # r,z gates
                    gn_sb = pc1.tile([FPC, 4, 512], BF16, tag="gn")   # inn
                    s1_sb = pc1.tile([FPC, D], BF16)
                    out32 = pc1.tile([FPC, D], F32)

                    def gi_pass(xt, gh_src):
                        """gi = wih x + bih (x64); g = gi/64 + gh for r,z; gi/64 for n."""
                        for jp in range(3):
                            pci_ps = pcps.tile([128, 512], F32, tag="pch")
                            wts = []
                            for g in range(4):
                                j = jp * 4 + g
                                wt = gsi_pre.pop(j, None)
                                if wt is None:
                                    wt = pci.tile([128, KC, 512], F8E3, tag="ci")
                                    nc.scalar.dma_start(out=wt, in_=gsid.ap()[j])
                                wts.append(wt)
                            for kc in range(KC):
                                for g in range(4):
                                    nc.tensor.matmul(pci_ps[32 * g:32 * g + 16, :],
                                                     lhsT=xt[:, kc, :], rhs=wts[g][:, kc, :],
                                                     tile_position=(0, 32 * g),
                                                     start=(kc == 0), stop=False, skip_group_check=True)
                            for g in range(4):
                                j = jp * 4 + g
                                bsi = pbias.tile([1, 512], BF16, tag="bias")
                                nc.sync.dma_start(out=bsi, in_=gsibd.ap()[:, j * 512:(j + 1) * 512])
                                nc.tensor.matmul(pci_ps[32 * g:32 * g + 16, :], lhsT=ones_b[0:1, 0:16],
                                                 rhs=bsi, tile_position=(0, 32 * g),
                                                 start=False, stop=True, skip_group_check=True)
                            for g in range(4):
                                j = jp * 4 + g
                                if j < 8:
                                    nc.vector.scalar_tensor_tensor(
                                        g_sb[:, j, :], pci_ps[32 * g:32 * g + 16, :], 1.0 / WS3,
                                        gh_src[:, j, :], op0=ALU.mult, op1=ALU.add)
                                else:
                                    nc.scalar.activation(gn_sb[:, j - 8, :], pci_ps[32 * g:32 * g + 16, :],
                                                         AF.Copy, scale=1.0 / WS3)

                    def s_elementwise(gh_src, hprev, outt):
                        for t in range(4):
                            cols = slice(t * 512, (t + 1) * 512)
                            r1 = pc1.tile([FPC, 512], F32, tag="c_r")
                            nc.scalar.activation(r1, g_sb[:, t, :], AF.Sigmoid)
                            z1 = pc1.tile([FPC, 512], F32, tag="c_z")
                            nc.scalar.activation(z1, g_sb[:, 4 + t, :], AF.Sigmoid)
                            u1 = pct.tile([FPC, 512], F32, tag="cu")
                            nc.vector.tensor_tensor(u1, r1, gh_src[:, 8 + t, :], op=ALU.mult)
                            u2 = pct.tile([FPC, 512], F32, tag="cu")
                            nc.vector.tensor_tensor(u2, u1, gn_sb[:, t, :], op=ALU.add)
                            n1 = pc1.tile([FPC, 512], F32, tag="c_n")
                            nc.scalar.activation(n1, u2, AF.Tanh)
                            u3 = pct.tile([FPC, 512], F32, tag="cu")
                            nc.vector.tensor_tensor(u3, hprev[:, cols], n1, op=ALU.subtract)
                            u4 = pct.tile([FPC, 512], F32, tag="cu")
                            nc.vector.tensor_tensor(u4, z1, u3, op=ALU.mult)
                            nc.vector.tensor_tensor(outt[:, cols], n1, u4, op=ALU.add)

                    gi_pass(ah_sb, gh1_sb)
                    s_elementwise(gh1_sb, sc4rm_sb, s1_sb)
                    for c in range(KC):
                        ptp = pctps.tile([128, 16], BF16, tag="tp")
                        nc.tensor.transpose(ptp, s1_sb[:, c * 128:(c + 1) * 128], ident16)
                        nc.scalar.copy(s1t_sb[:, c, :], ptp)
                    gi_pass(s1t_sb, gh2_sb)
                    s_elementwise(gh2_sb, sfrm_sb, out32)
                    nc.sync.dma_start(out=outp.ap(), in_=out32)

    nc.compile()
    return nc


def _tile_w(WT, blocks):
    """[2048, blocks*512] -> [blocks, 128, 16, 512] (pre-tiled for contiguous DMA)."""
    return np.ascontiguousarray(
        WT.reshape(16, 128, blocks, 512).transpose(2, 1, 0, 3))


def _tile_k(WT):
    """[2048, N] -> [128, 16, N]."""
    n = WT.shape[1]
    return np.ascontiguousarray(WT.reshape(16, 128, n).transpose(1, 0, 2))


def _make_hind():
    """h-broadcast indicator: hind[p, f*128+h*16+o] = (p%64 == f*16+o)."""
    m = np.zeros((128, 512), dtype=NB)
    for n in range(512):
        f, o = n // 128, n % 16
        m[f * 16 + o, n] = 1.0
        m[64 + f * 16 + o, n] = 1.0
    return m


def _prep_in_maps(inputs):
    E = np.ascontiguousarray(inputs["H_O_edges"].reshape(NFRAMES, ROWS, D))
    On = inputs["O_nodes"].reshape(NFRAMES, O, D)
    Hn = inputs["H_nodes"].reshape(NFRAMES, H, D)
    Sc4 = inputs["S_node_C4"].reshape(NFRAMES, D)
    Sf = np.ascontiguousarray(inputs["final_S_node"].transpose(0, 2, 1)).reshape(NFRAMES, D)

    We, Wl1, Wn = inputs["We"], inputs["Wl1"], inputs["Wn"]

    shared = {
        "wcatA": _tile_k((We * WS).T.astype(NE4)),
        "wcatB": _tile_k((Wl1 * WS).T.astype(NE4)),
        "wl1l": np.ascontiguousarray(
            (Wl1[:, :D // 2] * WS).T.astype(NE4).reshape(8, 128, D // 2).transpose(1, 0, 2)),
        "wl1r": np.ascontiguousarray(
            (Wl1[:, D // 2:] * WS).T.astype(NE4).reshape(8, 128, D // 2).transpose(1, 0, 2)),
        "wnt": _tile_k((Wn * WS).T.astype(NE4)),
        "wl2": np.ascontiguousarray(inputs["Wl2"][0].reshape(8, 128).T).astype(NB),
        "bl1t": np.ascontiguousarray(inputs["bl1"].reshape(8, 128).T).astype(NB),
        "bett": np.ascontiguousarray(inputs["be"].reshape(8, 128).T).astype(NB),
        "bntt": np.ascontiguousarray(inputs["bn"].reshape(8, 128).T).astype(NB),
        "hind": _make_hind(),
        "pmat": np.ascontiguousarray(np.kron(np.eye(FPC), np.ones((H, 1))) / H).astype(NB),
        "ghi": _tile_w((inputs["gh_wih"] * (WS / O)).T.astype(NE4), 12),
        "ghh": _tile_w((inputs["gh_whh"] * WS).T.astype(NE4), 12),
        "ghib": (inputs["gh_bih"] * WS)[None, :].astype(NB),
        "ghhb": (inputs["gh_bhh"] * WS)[None, :].astype(NB),
        "gsi": _tile_w((inputs["gs_wih"] * WS3).T.astype(NE3), 12),
        "gsh": _tile_w((inputs["gs_whh"] * WS3).T.astype(NE3), 12),
        "gsib": (inputs["gs_bih"] * WS3)[None, :].astype(NB),
        "gshb": (inputs["gs_bhh"] * WS3)[None, :].astype(NB),
    }

    in_maps = []
    for c in range(NCORES):
        fr = slice(c * FPC, (c + 1) * FPC)
        Ec = E[fr]  # [16, 128, 2048]
        e0t = np.ascontiguousarray(
            Ec.reshape(NQ, 4, ROWS, D).transpose(0, 3, 1, 2)
            .reshape(NQ, 16, 128, 512).transpose(0, 2, 1, 3)).astype(NE4)
        m = dict(shared)
        m.update({
            "e0t": e0t,
            "ot": _tile_k(On[fr].reshape(FPC * O, D).T.astype(NE4)),
            "ht8": _tile_k(Hn[fr].reshape(FPC * H, D).T.astype(NE4)),
            "h_rm": np.ascontiguousarray(Hn[fr].reshape(FPC * H, D)).astype(np.float32),
            "scsf": _tile_k(np.concatenate([Sc4[fr].T, Sf[fr].T], axis=1).astype(NB)),
            "sc4rm": np.ascontiguousarray(Sc4[fr]).astype(np.float32),
            "sfrm": np.ascontiguousarray(Sf[fr]).astype(np.float32),
        })
        in_maps.append(m)
    return in_maps


LAST_RESULT = None


def kernel(**inputs):
    global LAST_RESULT
    if "nc" not in _CACHE:
        _CACHE["nc"] = _build_nc()
    nc = _CACHE["nc"]
    in_maps = _prep_in_maps(inputs)
    trace = os.environ.get("KERNEL_TRACE", "0") == "1"
    res = bass_utils.run_bass_kernel_spmd(
        nc, in_maps, core_ids=list(range(NCORES)), trace=trace)
    LAST_RESULT = res
    out = np.concatenate([res.results[c]["outp"] for c in range(NCORES)], axis=0)
    return np.ascontiguousarray(out.reshape(B, F, D)).astype(np.float32)
